# revision 14
# baseline (speedup 1.0000x reference)
"""Trainium2 Bass kernel for Euclidean-ALiBi self-attention.

Problem: b=2, n=4096, d=512, h=8 heads (hd=64).
  q,k,v = x@W* + b*;  bias[b,h,i,j] = -slope_h * ||coords_i - coords_j||
  out = softmax(q k^T / sqrt(hd) + bias) v @ Wp + bp

Sharding: 8 cores = 2 batches x 4 query-chunks of 1024.  Each core computes
its full-depth attention for its query range against all 4096 keys (k/v
projections are recomputed per core -- cheaper than cross-core reduction),
and returns yT [512, 1024].  Host-side unshard is a pure transpose+concat.

Device layout per core (scores kept transposed: [keys_part, q_free]):
  - xT (d-major) streamed in bf16; k^T = Wk^T@xT, q^T = Wq^T@xTq (with the
    1/(8*slope_h) scale and bias row folded into host-prepped weights),
    v = x@Wv via xT-stationary matmuls.
  - dist^2 via K=8 augmented-coords matmul; dist = ACT sqrt(d2 + eps).
  - per head: S' = k^T q (bf16, K=128 via zero-padded per-head q^T),
    DVE fuse: arg = (S' - dist)*slope_h, ACT exp -> A (bf16),
    PV: out[q,65] += A_blk^T v_blk with a ones-column in v for the softmax
    denominator; normalize via ACT per-partition scale on the PSUM copy.
  - o transposed via PE; y^T = Wp^T o^T + bp; DMA out.
"""

import math
import os
import sys
from contextlib import ExitStack

for _p in ("/opt/trn_rl_repo", "/root/.axon_site/_ro/trn_rl_repo"):
    if os.path.isdir(_p) and _p not in sys.path:
        sys.path.insert(0, _p)

import numpy as np
import ml_dtypes

import concourse.bass as bass
import concourse.bacc as bacc
import concourse.mybir as mybir
import concourse.tile as tile
from concourse.bass_utils import run_bass_kernel_spmd
from concourse.masks import make_identity

F32 = mybir.dt.float32
BF16 = mybir.dt.bfloat16
P = 128
D = 512
H = 8
HD = 64
DC = D // P  # 4 d-chunks
EPS = 4e-3


def get_slopes(n):
    def pow2(n):
        start = 2 ** (-(2 ** (-(math.log2(n) - 3))))
        return [start * start**i for i in range(n)]

    if math.log2(n).is_integer():
        return pow2(n)
    c = 2 ** math.floor(math.log2(n))
    return pow2(c) + get_slopes(2 * c)[0::2][: n - c]


SLOPES = get_slopes(H)


def build_bass(NK, QCORE, QC, SG, D2G):
    """Build the per-core Bass program (SPMD: same program, per-core data)."""
    KB = NK // P          # key blocks
    NQC = QCORE // QC     # q-chunks per core
    QB = QC // P          # 128-q blocks per chunk
    NSG = KB // SG        # strip groups per chunk
    NCH = NK // 512       # 512-wide column chunks of NK
    QNCH = QCORE // 512   # 512-wide column chunks of QCORE
    assert KB * P == NK and NQC * QC == QCORE and QB * P == QC
    assert NSG * SG == KB and SG % D2G == 0 and KB % D2G == 0

    nc = bacc.Bacc()

    xT_d = nc.dram_tensor("xT", [D, NK], BF16, kind="ExternalInput")
    xTo_d = nc.dram_tensor("xTones", [P, NK], BF16, kind="ExternalInput")
    xTq_d = nc.dram_tensor("xTq", [D, QCORE], BF16, kind="ExternalInput")
    xTqo_d = nc.dram_tensor("xTqones", [P, QCORE], BF16, kind="ExternalInput")
    Wq_d = nc.dram_tensor("Wq", [D, D], BF16, kind="ExternalInput")
    Wqb_d = nc.dram_tensor("Wqb", [P, D], BF16, kind="ExternalInput")
    Wk_d = nc.dram_tensor("Wk", [D, D], BF16, kind="ExternalInput")
    Wkb_d = nc.dram_tensor("Wkb", [P, D], BF16, kind="ExternalInput")
    Wv_d = nc.dram_tensor("Wv", [D, D], BF16, kind="ExternalInput")
    Wvb_d = nc.dram_tensor("Wvb", [P, D], BF16, kind="ExternalInput")
    Wp_d = nc.dram_tensor("Wp", [D, D], BF16, kind="ExternalInput")
    bp_d = nc.dram_tensor("bp", [D, 1], F32, kind="ExternalInput")
    ak_d = nc.dram_tensor("ak", [P, NK], F32, kind="ExternalInput")
    bq_d = nc.dram_tensor("bq", [P, QCORE], F32, kind="ExternalInput")
    yT_d = nc.dram_tensor("yT", [D, QCORE], F32, kind="ExternalOutput")

    Exp = mybir.ActivationFunctionType.Exp
    Sqrt = mybir.ActivationFunctionType.Sqrt
    Ident = mybir.ActivationFunctionType.Identity
    sub = mybir.AluOpType.subtract
    amax = mybir.AluOpType.max

    with ExitStack() as ctx:
        tc = ctx.enter_context(tile.TileContext(nc))
        persist = ctx.enter_context(tc.tile_pool(name="persist", bufs=1))
        pmisc = ctx.enter_context(tc.tile_pool(name="psmisc", bufs=2, space="PSUM"))

        # ---- persistent tiles ----
        kT = persist.tile([P, DC, NK], BF16, name="kT")
        qT = persist.tile([P, H, QCORE], BF16, name="qT")
        vext = persist.tile([P, KB, H, HD + 1], BF16, name="vext")
        oT = persist.tile([P, DC, QCORE], BF16, name="oT")
        Wp_sb = persist.tile([P, DC, D], BF16, name="Wp")
        bp_sb = persist.tile([P, DC], F32, name="bp")
        ident = persist.tile([P, P], BF16, name="ident")
        eps_t = persist.tile([P, 1], F32, name="eps")
        ak = persist.tile([P, NK], F32, name="ak")
        bq = persist.tile([P, QCORE], F32, name="bq")

        make_identity(nc, ident[:])
        nc.gpsimd.memset(eps_t[:], EPS)
        nc.vector.memset(qT[:], 0.0)
        nc.sync.dma_start(ak[:], ak_d[:])
        nc.sync.dma_start(bq[:], bq_d[:])
        for c in range(DC):
            nc.sync.dma_start(Wp_sb[:, c, :], Wp_d[c * P : (c + 1) * P, :])
            nc.sync.dma_start(bp_sb[:, c : c + 1], bp_d[c * P : (c + 1) * P, :])

        # ---- setup phase: projections ----
        with tc.tile_pool(name="setup", bufs=1) as sp:
            xT = sp.tile([P, DC, NK], BF16, name="xT")
            xTo = sp.tile([P, NK], BF16, name="xTo")
            xTq = sp.tile([P, DC, QCORE], BF16, name="xTq")
            xTqo = sp.tile([P, QCORE], BF16, name="xTqo")
            Wq = sp.tile([P, DC, D], BF16, name="Wq")
            Wk = sp.tile([P, DC, D], BF16, name="Wk")
            Wv = sp.tile([P, DC, D], BF16, name="Wv")
            Wqb = sp.tile([P, D], BF16, name="Wqb")
            Wkb = sp.tile([P, D], BF16, name="Wkb")
            Wvb = sp.tile([P, D], BF16, name="Wvb")

            for c in range(DC):
                nc.sync.dma_start(xT[:, c, :], xT_d[c * P : (c + 1) * P, :])
                nc.sync.dma_start(xTq[:, c, :], xTq_d[c * P : (c + 1) * P, :])
                nc.sync.dma_start(Wq[:, c, :], Wq_d[c * P : (c + 1) * P, :])
                nc.sync.dma_start(Wk[:, c, :], Wk_d[c * P : (c + 1) * P, :])
                nc.sync.dma_start(Wv[:, c, :], Wv_d[c * P : (c + 1) * P, :])
            nc.sync.dma_start(xTo[:], xTo_d[:])
            nc.sync.dma_start(xTqo[:], xTqo_d[:])
            nc.sync.dma_start(Wqb[:], Wqb_d[:])
            nc.sync.dma_start(Wkb[:], Wkb_d[:])
            nc.sync.dma_start(Wvb[:], Wvb_d[:])

            # kT[dkb] = Wk[:, dkb]^T @ xT  (+ bias row via ones chunk)
            for dkb in range(DC):
                for nch in range(NCH):
                    ps = pmisc.tile([P, 512], F32, tag="ps512")
                    for c in range(DC):
                        nc.tensor.matmul(
                            ps[:],
                            lhsT=Wk[:, c, dkb * P : (dkb + 1) * P],
                            rhs=xT[:, c, nch * 512 : (nch + 1) * 512],
                            start=(c == 0),
                            stop=False,
                        )
                    nc.tensor.matmul(
                        ps[:],
                        lhsT=Wkb[:, dkb * P : (dkb + 1) * P],
                        rhs=xTo[:, nch * 512 : (nch + 1) * 512],
                        start=False,
                        stop=True,
                    )
                    nc.scalar.copy(kT[:, dkb, nch * 512 : (nch + 1) * 512], ps[:])

            # qT (pair-padded per head; scale folded into host Wq)
            for dkb in range(DC):
                for nch in range(QNCH):
                    ps = pmisc.tile([P, 512], F32, tag="ps512")
                    for c in range(DC):
                        nc.tensor.matmul(
                            ps[:],
                            lhsT=Wq[:, c, dkb * P : (dkb + 1) * P],
                            rhs=xTq[:, c, nch * 512 : (nch + 1) * 512],
                            start=(c == 0),
                            stop=False,
                        )
                    nc.tensor.matmul(
                        ps[:],
                        lhsT=Wqb[:, dkb * P : (dkb + 1) * P],
                        rhs=xTqo[:, nch * 512 : (nch + 1) * 512],
                        start=False,
                        stop=True,
                    )
                    sl = slice(nch * 512, (nch + 1) * 512)
                    nc.scalar.copy(qT[0:HD, 2 * dkb, sl], ps[0:HD, :])
                    nc.scalar.copy(qT[HD:P, 2 * dkb + 1, sl], ps[HD:P, :])

            # v = x @ Wv  (natural layout, per key block) + ones column
            for kb in range(KB):
                ps = pmisc.tile([P, 512], F32, tag="ps512")
                for c in range(DC):
                    nc.tensor.matmul(
                        ps[:],
                        lhsT=xT[:, c, kb * P : (kb + 1) * P],
                        rhs=Wv[:, c, :],
                        start=(c == 0),
                        stop=False,
                    )
                nc.tensor.matmul(
                    ps[:],
                    lhsT=xTo[:, kb * P : (kb + 1) * P],
                    rhs=Wvb[:],
                    start=False,
                    stop=True,
                )
                nc.vector.memset(vext[:, kb, :, HD : HD + 1], 1.0)
                nc.vector.tensor_copy(
                    vext[:, kb, :, 0:HD],
                    ps[:].rearrange("p (h d) -> p h d", h=H),
                )

        # ---- main phase ----
        mainp = ctx.enter_context(tc.tile_pool(name="mainp", bufs=1))
        dist = mainp.tile([P, KB, QC], F32, name="dist")
        argp = ctx.enter_context(tc.tile_pool(name="argp", bufs=2))
        Apool = ctx.enter_context(tc.tile_pool(name="Apool", bufs=2))
        osb = ctx.enter_context(tc.tile_pool(name="osb", bufs=3))
        scr = ctx.enter_context(tc.tile_pool(name="scr", bufs=4))
        ps4 = ctx.enter_context(tc.tile_pool(name="ps4", bufs=2, space="PSUM"))
        pso = ctx.enter_context(tc.tile_pool(name="pso", bufs=1, space="PSUM"))

        for qc in range(NQC):
            qs = slice(qc * QC, (qc + 1) * QC)

            # dist strip for this q-chunk (shared by all heads)
            for g in range(KB // D2G):
                pd = ps4.tile([P, D2G, QC], F32, tag="ps4")
                for j in range(D2G):
                    kb = g * D2G + j
                    nc.tensor.matmul(
                        pd[:, j, :],
                        lhsT=ak[:, kb * P : (kb + 1) * P],
                        rhs=bq[:, qs],
                        start=True,
                        stop=True,
                    )
                nc.scalar.activation(
                    dist[:, g * D2G : (g + 1) * D2G, :], pd[:], Sqrt, bias=eps_t[:]
                )

            o_t = [
                osb.tile([P, D], BF16, tag="osb", name=f"o_{qc}_{qb}")
                for qb in range(QB)
            ]

            for h in range(H):
                po = [
                    pso.tile([P, HD + 1], F32, tag=f"po{qb}", name=f"po_{qc}_{h}_{qb}")
                    for qb in range(QB)
                ]
                for sg in range(NSG):
                    arg = argp.tile([P, SG, QC], F32, tag="arg")
                    for g in range(SG // D2G):
                        ps = ps4.tile([P, D2G, QC], F32, tag="ps4")
                        for j in range(D2G):
                            kb = sg * SG + g * D2G + j
                            nc.tensor.matmul(
                                ps[:, j, :],
                                lhsT=kT[:, h // 2, kb * P : (kb + 1) * P],
                                rhs=qT[:, h, qs],
                                start=True,
                                stop=True,
                            )
                        nc.vector.tensor_tensor(
                            out=arg[:, g * D2G : (g + 1) * D2G, :],
                            in0=ps[:],
                            in1=dist[:, sg * SG + g * D2G : sg * SG + (g + 1) * D2G, :],
                            op=sub,
                        )
                    A = Apool.tile([P, SG, QC], BF16, tag="A")
                    # exp(slope_h * (S' - dist)) -- slope folded into ACT scale
                    nc.scalar.activation(A[:], arg[:], Exp, scale=float(SLOPES[h]))
                    for j in range(SG):
                        kb = sg * SG + j
                        for qb in range(QB):
                            nc.tensor.matmul(
                                po[qb][:],
                                lhsT=A[:, j, qb * P : (qb + 1) * P],
                                rhs=vext[:, kb, h, :],
                                start=(sg == 0 and j == 0),
                                stop=(sg == NSG - 1 and j == SG - 1),
                            )
                # normalize: o = po[:, :HD] / po[:, HD]
                r = scr.tile([P, QB], F32, tag="r")
                for qb in range(QB):
                    nc.vector.reciprocal(r[:, qb : qb + 1], po[qb][:, HD : HD + 1])
                    nc.vector.tensor_scalar_mul(
                        o_t[qb][:, h * HD : (h + 1) * HD],
                        po[qb][:, 0:HD],
                        r[:, qb : qb + 1],
                    )

            # transpose o -> oT columns for this q-chunk
            for qb in range(QB):
                for fb in range(DC):
                    pt = pmisc.tile([P, 1024], BF16, tag="ps512")
                    nc.tensor.transpose(
                        pt[:, 0:P], o_t[qb][:, fb * P : (fb + 1) * P], ident[:]
                    )
                    cs = slice(qc * QC + qb * P, qc * QC + (qb + 1) * P)
                    nc.vector.tensor_copy(oT[:, fb, cs], pt[:, 0:P])

        # ---- final projection: yT = Wp^T oT + bp ----
        for ob in range(DC):
            for nch in range(QNCH):
                py = pmisc.tile([P, 512], F32, tag="ps512")
                for fb in range(DC):
                    nc.tensor.matmul(
                        py[:],
                        lhsT=Wp_sb[:, fb, ob * P : (ob + 1) * P],
                        rhs=oT[:, fb, nch * 512 : (nch + 1) * 512],
                        start=(fb == 0),
                        stop=(fb == DC - 1),
                    )
                ysb = osb.tile([P, 512], F32, tag="ysb")
                nc.scalar.activation(ysb[:], py[:], Ident, bias=bp_sb[:, ob : ob + 1])
                nc.sync.dma_start(
                    yT_d[ob * P : (ob + 1) * P, nch * 512 : (nch + 1) * 512], ysb[:]
                )

    nc.compile()
    return nc


def host_prep_core(x, coords, Wq, bq_b, Wk, bk, Wv, bv, Wp, bp, NK, QCORE, qlo):
    """Build one core's input map. x: [n, d] (this core's batch), coords: [n, 2]."""
    bf = ml_dtypes.bfloat16
    xT = np.ascontiguousarray(x.T)
    colscale = np.repeat(1.0 / (8.0 * np.array(SLOPES, np.float64)), HD)

    def aug(W, b, scale=None):
        Wf = W.astype(np.float64)
        bf_ = b.astype(np.float64)
        if scale is not None:
            Wf = Wf * scale[None, :]
            bf_ = bf_ * scale
        Wb = np.zeros((128, D), np.float64)
        Wb[0] = bf_
        return Wf.astype(bf), Wb.astype(bf)

    Wq_s, Wqb_s = aug(Wq, bq_b, colscale)
    Wk_s, Wkb_s = aug(Wk, bk)
    Wv_s, Wvb_s = aug(Wv, bv)

    ones_nk = np.zeros((128, NK), np.float32)
    ones_nk[0] = 1.0
    ones_qc = np.zeros((128, QCORE), np.float32)
    ones_qc[0] = 1.0

    c = coords - 50.0
    ck = c  # keys: all n
    cq = c[qlo : qlo + QCORE]
    ak = np.zeros((128, NK), np.float32)
    ak[0] = 1.0
    ak[1] = ck[:, 0]
    ak[2] = ck[:, 1]
    ak[3] = (ck * ck).sum(-1)
    bq = np.zeros((128, QCORE), np.float32)
    bq[0] = (cq * cq).sum(-1)
    bq[1] = -2.0 * cq[:, 0]
    bq[2] = -2.0 * cq[:, 1]
    bq[3] = 1.0

    return {
        "xT": xT.astype(bf),
        "xTones": ones_nk.astype(bf),
        "xTq": np.ascontiguousarray(xT[:, qlo : qlo + QCORE]).astype(bf),
        "xTqones": ones_qc.astype(bf),
        "Wq": Wq_s,
        "Wqb": Wqb_s,
        "Wk": Wk_s,
        "Wkb": Wkb_s,
        "Wv": Wv_s,
        "Wvb": Wvb_s,
        "Wp": Wp.astype(bf),
        "bp": bp.reshape(D, 1).astype(np.float32),
        "ak": ak,
        "bq": bq,
    }


_NC_CACHE = {}
LAST_RESULT = None
RUN_KWARGS = {}


def _get_nc(NK, QCORE, QC, SG, D2G):
    key = (NK, QCORE, QC, SG, D2G)
    if key not in _NC_CACHE:
        _NC_CACHE[key] = build_bass(NK, QCORE, QC, SG, D2G)
    return _NC_CACHE[key]


def kernel(x, coords, Wq, bq, Wk, bk, Wv, bv, Wp, bp, slopes, **_):
    x = np.asarray(x, np.float32)
    coords = np.asarray(coords, np.float32)
    B, N, _d = x.shape
    NK, QCORE, QC, SG, D2G = N, N // 4, 256, 16, 4
    nc = _get_nc(NK, QCORE, QC, SG, D2G)

    in_maps = []
    for core in range(8):
        b, qi = core // 4, core % 4
        in_maps.append(
            host_prep_core(
                x[b], coords[b],
                np.asarray(Wq, np.float32), np.asarray(bq, np.float32),
                np.asarray(Wk, np.float32), np.asarray(bk, np.float32),
                np.asarray(Wv, np.float32), np.asarray(bv, np.float32),
                np.asarray(Wp, np.float32), np.asarray(bp, np.float32),
                NK, QCORE, qi * QCORE,
            )
        )

    global LAST_RESULT
    res = run_bass_kernel_spmd(nc, in_maps, core_ids=list(range(8)), **RUN_KWARGS)
    LAST_RESULT = res
    out = np.empty((B, N, D), np.float32)
    for core in range(8):
        b, qi = core // 4, core % 4
        out[b, qi * QCORE : (qi + 1) * QCORE, :] = res.results[core]["yT"].T
    return out
